# revision 13
# baseline (speedup 1.0000x reference)
"""DirectionalConv3d Trainium2 kernel.

out[b, o, t, r, c] = sum_d W_d[o, :] . x[b, :, (t,r,c)+delta_d]
for the 7-point directional stencil (self, t+-1, r+-1, c+-1), zero padded.

Strategy (per core, 1 batch per core, 8 cores):
  - Keep x in native [C_in=64, T*R*C] layout: channels on SBUF partitions,
    so out^T = W @ x is a direct matmul and spatial shifts are free-dim
    address offsets.  No transposes anywhere.
  - Split the volume across the partition axis: partitions 0-63 hold x
    planes 0..16 and compute output planes 0..15, partitions 64-127 hold
    x planes 15..31 and compute output planes 16..31.  The two halves run
    as concurrent matmuls on PE row-groups (tile_position (0,0)/(64,64)).
  - Per output plane and per 512-column half (one PSUM bank): 7
    accumulating matmuls, one per direction, rhs AP shifted by the
    stencil offset.  r+-1 shifts trim rows off the (contiguous) psum AP;
    t+-1 shifts skip at volume ends; c+-1 shifts read the dense image
    with wrap-around and two tiny stride-32 correction matmuls with
    negated weights subtract the wrapped columns exactly.
  - DMA-in casts f32 -> bf16 in flight (SWDGE) straight into the dense
    image.  Weights are pre-cast/pre-transposed host-side (tiny, with
    -W_cp / -W_cm appended for the corrections).  PSUM is evacuated by
    VectorE and ScalarE alternately into SBUF staging, then DMA'd out as
    f32.
"""

import numpy as np
import ml_dtypes

B = 8
CI = 64
CO = 64
T = 32
R = 32
C = 32
PL = R * C          # 1024 elements per plane
N = T * PL          # 32768 elements per channel in HBM
HALF_T = 17         # image planes per partition-half (with 1-plane halo)
BOT = 15            # bottom half holds x planes BOT .. BOT+16
LEAD = 1            # zero guard element before/after each chunk's data
CHUNKS = [(0, 6), (6, 6), (12, 5)]   # (first image plane, n planes) per chunk

# direction -> (dt, dr, dc): out[t,r,c] += W_d . x[t+dt, r+dr, c+dc]
DIRS = [
    ("self", 0, 0, 0),
    ("tp", -1, 0, 0),
    ("tm", 1, 0, 0),
    ("rp", 0, -1, 0),
    ("rm", 0, 1, 0),
    ("cp", 0, 0, -1),
    ("cm", 0, 0, 1),
]
NW = 9              # 7 directions + negated cp/cm for corrections

X_DT_NAME = "bfloat16"   # dtype of x image + weights in SBUF

_NC_CACHE = {}


def _chunk_of(plane):
    for ci, (p0, np_) in enumerate(CHUNKS):
        if p0 <= plane < p0 + np_:
            return ci, p0, np_
    raise AssertionError(plane)


def _emit(nc, tc, x, wt, out, mybir, bass):
    xdt = getattr(mybir.dt, X_DT_NAME)
    AP = bass.AP

    xpool = tc.alloc_tile_pool(name="xin", bufs=1)
    wpool = tc.alloc_tile_pool(name="wp", bufs=1)
    apool = tc.alloc_tile_pool(name="accp", bufs=4, space="PSUM")
    spool = tc.alloc_tile_pool(name="stg", bufs=3)

    # ---- weights [9,64,64]: lhsT layout host-side, incl. -cp/-cm ----
    w_sb = wpool.tile([128, NW * CO], xdt, name="w_sb")
    w_src = wt.transpose([1, 0, 2])  # [i, d, o]
    nc.sync.dma_start(out=w_sb[0:64, :], in_=w_src)
    nc.sync.dma_start(out=w_sb[64:128, :], in_=w_src)

    # ---- x image chunks (dense, with 1 zero guard element each side) ----
    xts = []
    for ci, (p0, np_) in enumerate(CHUNKS):
        xt = xpool.tile([128, np_ * PL + 2 * LEAD], xdt, name=f"xc{ci}")
        src = AP(x.tensor, p0 * PL, [[BOT * PL, 2], [N, CI], [1, np_ * PL]])
        nc.gpsimd.dma_start(out=xt[:, LEAD:LEAD + np_ * PL], in_=src)  # casts
        nc.vector.memset(xt[:, 0:LEAD], 0.0)
        nc.vector.memset(xt[:, LEAD + np_ * PL:], 0.0)
        xts.append(xt)

    def w_ap(di, h):
        return w_sb[h * 64:(h + 1) * 64, di * CO:(di + 1) * CO]

    def img(h, plane):
        """(tile, base offset) of an image plane for partition-half h."""
        ci, p0, np_ = _chunk_of(plane)
        return xts[ci], LEAD + (plane - p0) * PL

    # ---- main loop over plane pairs ----
    stage = None
    for pp in range(T // 2):
        accs = []
        for j in range(2):
            acc = apool.tile([128, 512], mybir.dt.float32, name=f"acc{pp}_{j}",
                             tag="acc")
            accs.append(acc)

        for di, (dname, dt_, dr, dc) in enumerate(DIRS):
            for h in range(2):
                t_out = pp if h == 0 else pp + 16
                spa = t_out + dt_
                if not (0 <= spa < T):
                    continue
                xt, base = img(h, spa if h == 0 else spa - BOT)
                lo = h * 64
                w = w_ap(di, h)
                for j in range(2):
                    r0 = j * 16
                    or0 = max(r0, -dr)      # out rows valid iff 0<=r+dr<R
                    or1 = min(r0 + 16, R - dr)
                    oap = accs[j][lo:lo + 64, (or0 - r0) * C:(or1 - r0) * C]
                    s0 = base + (or0 + dr) * C + dc
                    rhs = xt[lo:lo + 64, s0: s0 + (or1 - or0) * C]
                    nc.tensor.matmul(
                        out=oap, lhsT=w, rhs=rhs,
                        start=(di == 0), stop=False,
                        # the sim's psum-group tracker aliases partition
                        # ranges within a bank; has_written is per-element
                        # on HW so split-bank start/accumulate is safe
                        skip_group_check=True,
                    )

        # correction matmuls: subtract the wrapped column contributions
        # cp polluted out[.., r, 0] with W_cp.x[prev elem]; cm polluted
        # out[.., r, 31] with W_cm.x[next elem].
        for h in range(2):
            t_out = pp if h == 0 else pp + 16
            xt, base = img(h, t_out if h == 0 else t_out - BOT)
            lo = h * 64
            for j in range(2):
                r0 = j * 16
                # cp correction: out col 0, rows r0..r0+15
                oap = accs[j][lo:lo + 64, 0:481:32]
                s0 = base + r0 * C - 1
                rhs = xt[lo:lo + 64, s0:s0 + 481:32]
                nc.tensor.matmul(out=oap, lhsT=w_ap(7, h), rhs=rhs,
                                 start=False, stop=False,
                                 skip_group_check=True)
                # cm correction: out col 31
                oap = accs[j][lo:lo + 64, 31:512:32]
                s1 = base + (r0 + 1) * C
                rhs = xt[lo:lo + 64, s1:s1 + 481:32]
                nc.tensor.matmul(out=oap, lhsT=w_ap(8, h), rhs=rhs,
                                 start=False, stop=True,
                                 skip_group_check=True)

        # ---- evacuate PSUM -> SBUF staging; DMA out every 2 plane pairs ----
        if pp % 2 == 0:
            stage = spool.tile([128, 2 * PL], mybir.dt.float32, name=f"st{pp}",
                               tag="st")
        soff = (pp % 2) * PL
        nc.vector.tensor_copy(out=stage[:, soff:soff + 512], in_=accs[0][:, :])
        nc.scalar.copy(out=stage[:, soff + 512:soff + PL], in_=accs[1][:, :])
        if pp % 2 == 1:
            dst_top = AP(out.tensor, (pp - 1) * PL, [[N, CO], [1, 2 * PL]])
            dst_bot = AP(out.tensor, (pp + 15) * PL, [[N, CO], [1, 2 * PL]])
            nc.sync.dma_start(out=dst_top, in_=stage[0:64, :])
            nc.sync.dma_start(out=dst_bot, in_=stage[64:128, :])

    for p in (spool, apool, wpool, xpool):
        p.release()


def _split_multi_waits(nc, mybir):
    """Walrus codegen allows only one sem-wait slot per engine instruction
    ("Too many sync wait commands").  Hoist all but one wait of any
    multi-wait instruction onto InstNoOp's inserted immediately before it
    on the same engine queue — semantically identical for in-order
    engines (the nop blocks the queue until its wait passes).
    """
    SyncInfo = mybir.SyncInfo
    counter = [0]
    for blk in nc.m.functions[0].blocks:
        insts = list(blk.instructions)
        out, changed = [], False
        for inst in insts:
            si = getattr(inst, "sync_info", None)
            waits = list(si.on_wait) if si is not None and si.on_wait else []
            if len(waits) > 1:
                for w in waits[:-1]:
                    nop = mybir.InstNoOp(name=f"waitnop_{counter[0]}")
                    counter[0] += 1
                    nop.engine = inst.engine
                    nop.sync_info = SyncInfo(on_wait=[w], on_update=[])
                    nc.register_instruction(nop, overwrite=True)
                    out.append(nop)
                si.on_wait = [waits[-1]]
                changed = True
            out.append(inst)
        if changed:
            blk.instructions = out


def build_nc():
    import concourse.bass as bass
    import concourse.mybir as mybir
    import concourse.tile as tile

    key = X_DT_NAME
    if key in _NC_CACHE:
        return _NC_CACHE[key]
    nc = bass.Bass("TRN2", target_bir_lowering=False, debug=False)
    wdt = getattr(mybir.dt, X_DT_NAME)
    x = nc.dram_tensor("x", [CI, N], mybir.dt.float32, kind="ExternalInput").ap()
    wt = nc.dram_tensor("wt", [NW, CI, CO], wdt, kind="ExternalInput").ap()
    out = nc.dram_tensor("out", [CO, N], mybir.dt.float32,
                         kind="ExternalOutput").ap()
    with tile.TileContext(nc) as tc:
        _emit(nc, tc, x, wt, out, mybir, bass)
    _split_multi_waits(nc, mybir)
    _NC_CACHE[key] = nc
    return nc


def host_weights(inputs):
    """Stack + transpose the weights into lhsT layout [9, i, o].

    Entries 0-6 follow DIRS; 7 = -W_cp, 8 = -W_cm (wrap corrections).
    """
    ws = [np.asarray(inputs[n], dtype=np.float32)
          for n in ("w_self", "w_tp", "w_tm", "w_rp", "w_rm", "w_cp", "w_cm")]
    ws.append(-ws[5])
    ws.append(-ws[6])
    wt = np.stack([np.ascontiguousarray(w.T) for w in ws])
    if X_DT_NAME == "bfloat16":
        wt = wt.astype(ml_dtypes.bfloat16)
    return wt


def kernel(**inputs):
    from concourse.bass_utils import run_bass_kernel_spmd

    nc = build_nc()
    x = np.asarray(inputs["x"], dtype=np.float32)
    wt = host_weights(inputs)
    in_maps = [
        {"x": np.ascontiguousarray(x[b].reshape(CI, N)), "wt": wt}
        for b in range(B)
    ]
    res = run_bass_kernel_spmd(nc, in_maps, list(range(B))).results
    out = np.stack([res[b]["out"].reshape(CO, T, R, C) for b in range(B)])
    return out.astype(np.float32)
